# revision 9
# baseline (speedup 1.0000x reference)
"""Trainium2 Bass kernel for nn_EuclideanEmbedding (fused cutoff-multiply +
segment_sum over 3.2M edges into 100k nodes, 16 features).

Strategy (v2, "paged"):
Host: counting-sort edges by receiver, shard nodes across 8 cores
(12512 nodes = 782 16-node buckets per core), pad each bucket's edges to
cap = 128*cb slots, lay out chunk-major. Senders shipped in bf16,
lengths fp16, node-in-bucket key (+1) bf16.

Device (per core):
- w = 0.5/32 * (cos(pi*len/5)+1) * (len<5) computed once (ACT sin + DVE).
- For each block of 112 buckets: ONE is_equal against a static iota tile
  builds the one-hot sel[e, j, c] = (rk[e,c] == j) and ONE in-place
  multiply by broadcast w gives sel_w = w * one-hot. Both run in the DVE
  2x_1p mode (packed bf16; the broadcast sits on the middle dim, so the
  packed-last-dim requirement holds).
- Per chunk, one PE matmul (lhsT = x chunk [128,16] bf16, moving = sel_w
  [128,16]) accumulates node sums into PSUM. PE array col-tiling
  (tile_position) packs 4x28 buckets per [128, 448] PSUM tile.
- ACT drains PSUM straight into the output staging buffer.

Output rows >= 100000 of the full [3.2M, 16] result are identically zero
(receivers < 100000), assembled host-side.
"""
import math

import numpy as np

E = 3_200_000
F = 16
N_NODES = 100_000
R_CUT = 5.0
INV_AVG = 1.0 / 32.0

N_CORES = 8
W = 16                       # nodes per bucket
RBUCKETS = 782               # real buckets per core (782*16 = 12512 nodes)
NODES_PER_CORE = RBUCKETS * W
BLKS = 7                     # processing blocks per core
UPB = 112                    # bucket slots per block (4 PE col-tiles x 28)
BUCKETS_CORE = BLKS * UPB    # 784 padded bucket slots
NODES_PAD = BUCKETS_CORE * W
KK = UPB // 4                # col-ranges per PE col-tile (28)
PSUM_COLS = KK * W           # 448
N_RBUCKETS_TOT = (N_NODES + W - 1) // W   # 6250

_CACHE = {}


def _build_program(cb: int, reps: int = 1):
    """Build the Bass/Tile program for chunks-per-bucket `cb`.

    reps > 1 repeats the whole computation inside one NEFF -- used only for
    differential timing (amortizes the axon dispatch floor)."""
    from contextlib import ExitStack

    import concourse.bacc as bacc
    import concourse.tile as tile
    from concourse import mybir

    T = UPB * cb                    # chunks per block
    nchunks = BLKS * T

    nc = bacc.Bacc("TRN2", target_bir_lowering=False, debug=False,
                   enable_asserts=False, num_devices=N_CORES)
    x_dram = nc.dram_tensor("x_t", [BLKS, 128, T * F], mybir.dt.bfloat16,
                            kind="ExternalInput").ap()
    len_dram = nc.dram_tensor("len_t", [128, nchunks], mybir.dt.float16,
                              kind="ExternalInput").ap()
    rk_dram = nc.dram_tensor("rk_t", [128, nchunks], mybir.dt.bfloat16,
                             kind="ExternalInput").ap()
    out_dram = nc.dram_tensor("out", [NODES_PAD, F], mybir.dt.float32,
                              kind="ExternalOutput").ap()

    with tile.TileContext(nc) as tc, ExitStack() as ctx:
        small = ctx.enter_context(tc.tile_pool(name="small", bufs=1))
        xin = ctx.enter_context(tc.tile_pool(name="xin", bufs=3))
        work = ctx.enter_context(tc.tile_pool(name="work", bufs=3))
        psum = ctx.enter_context(tc.tile_pool(name="psum", bufs=4,
                                              space="PSUM"))

        leng = small.tile([128, nchunks], mybir.dt.float16)
        rkp1 = small.tile([128, nchunks], mybir.dt.bfloat16)
        nc.sync.dma_start(leng[:], len_dram[:])
        nc.sync.dma_start(rkp1[:], rk_dram[:])

        halfpi = small.tile([128, 1], mybir.dt.float32)
        nc.gpsimd.memset(halfpi[:], math.pi / 2)
        u = small.tile([128, nchunks], mybir.dt.float32)
        msk = small.tile([128, nchunks], mybir.dt.float32)
        wt = small.tile([128, nchunks], mybir.dt.bfloat16)
        # u = sin(pi/2 - (pi/R_CUT) len) = cos(pi len / R_CUT)
        nc.scalar.activation(u[:], leng[:], mybir.ActivationFunctionType.Sin,
                             bias=halfpi[:, 0:1], scale=-math.pi / R_CUT)
        nc.vector.tensor_scalar(msk[:], leng[:], R_CUT, None,
                                mybir.AluOpType.is_lt)
        nc.vector.tensor_scalar(u[:], u[:], 1.0, 0.5 * INV_AVG,
                                mybir.AluOpType.add, mybir.AluOpType.mult)
        nc.vector.tensor_tensor(wt[:], u[:], msk[:], mybir.AluOpType.mult)

        final = small.tile([128, BLKS * PSUM_COLS], mybir.dt.float32)

        # static iota: iota[p, j*T + c] = j
        iot = small.tile([128, W * T], mybir.dt.bfloat16)
        iot3 = iot[:].rearrange("p (j c) -> p j c", c=T)
        for j in range(W):
            nc.gpsimd.memset(iot3[:, j, :], float(j))

        for _rep in range(reps):
            for blk in range(BLKS):
                xt = xin.tile([128, T * F], mybir.dt.bfloat16)
                nc.sync.dma_start(xt[:], x_dram[blk])

                selw = work.tile([128, W * T], mybir.dt.bfloat16, tag="selw")
                selw3 = selw[:].rearrange("p (j c) -> p j c", c=T)
                w_bc = wt[:, blk * T:(blk + 1) * T].unsqueeze(1) \
                    .broadcast_to([128, W, T])
                rk_bc = rkp1[:, blk * T:(blk + 1) * T].unsqueeze(1) \
                    .broadcast_to([128, W, T])
                nc.vector.tensor_tensor(selw3, iot3, rk_bc,
                                        mybir.AluOpType.is_equal)
                nc.vector.tensor_tensor(selw3, selw3, w_bc,
                                        mybir.AluOpType.mult)

                pt = psum.tile([128, PSUM_COLS], mybir.dt.float32)
                for uu in range(UPB):
                    bb, kk = uu // KK, uu % KK
                    for ph in range(cb):
                        c = uu * cb + ph
                        nc.tensor.matmul(
                            out=pt[32 * bb:32 * bb + 16,
                                   W * kk:W * (kk + 1)],
                            lhsT=xt[:, F * c:F * (c + 1)],
                            rhs=selw3[:, :, c],
                            start=(ph == 0), stop=(ph == cb - 1),
                            tile_position=(0, 32 * bb))
                for bb in range(4):
                    nc.scalar.copy(
                        final[32 * bb:32 * bb + 16,
                              PSUM_COLS * blk:PSUM_COLS * (blk + 1)],
                        pt[32 * bb:32 * bb + 16, :])

        # out rows n = 16*(112*blk + 28*bb + kk) + j, feature f
        # <- final[32*bb + f, PSUM_COLS*blk + W*kk + j]; rows for a fixed
        # (blk, bb) are the contiguous span [1792*blk + 448*bb, +448)
        out4 = out_dram.rearrange("(blk bbx r) f -> blk bbx f r",
                                  bbx=4, r=PSUM_COLS)
        for blk in range(BLKS):
            for bb in range(4):
                nc.sync.dma_start(
                    out4[blk][bb],
                    final[32 * bb:32 * bb + 16,
                          PSUM_COLS * blk:PSUM_COLS * (blk + 1)])

    nc.compile()
    return nc


def _prepare_inputs(senders, lengths, receivers, cb: int):
    """Counting-sort + bucket-pad + tile-transpose. Returns in_maps (8 dicts)."""
    import ml_dtypes

    cap = cb * 128
    T = UPB * cb
    nchunks = BLKS * T

    recv = np.ascontiguousarray(np.asarray(receivers).astype(np.int64))
    order = np.argsort(recv, kind="stable").astype(np.int64)
    rs = recv[order]                          # sorted receivers
    rk_sorted = (rs % W).astype(np.float32)   # node-in-bucket key
    gbucket_counts = np.bincount((rs // W).astype(np.int64),
                                 minlength=N_RBUCKETS_TOT)
    starts = np.concatenate([[0], np.cumsum(gbucket_counts)[:-1]])

    senders_ext = np.concatenate(
        [np.asarray(senders, dtype=np.float32), np.zeros((1, F), np.float32)])
    senders_bf = senders_ext.astype(ml_dtypes.bfloat16)
    len_ext = np.concatenate(
        [np.asarray(lengths, dtype=np.float32).reshape(-1),
         np.full(1, 6.0, np.float32)]).astype(np.float16)
    rk_ext = np.concatenate([rk_sorted, np.full(1, -1.0, np.float32)]) \
        .astype(ml_dtypes.bfloat16)

    arange_cap = np.arange(cap, dtype=np.int64)

    in_maps = []
    for k in range(N_CORES):
        bidx = RBUCKETS * k + np.arange(BUCKETS_CORE)
        slot_real = (np.arange(BUCKETS_CORE) < RBUCKETS) & \
            (bidx < N_RBUCKETS_TOT)
        cnt = np.where(slot_real,
                       gbucket_counts[np.minimum(bidx, N_RBUCKETS_TOT - 1)], 0)
        st = np.where(slot_real,
                      starts[np.minimum(bidx, N_RBUCKETS_TOT - 1)], 0)
        if cnt.max() > cap:
            raise ValueError(f"bucket overflow: {cnt.max()} > {cap}")
        src = st[:, None] + arange_cap[None, :]       # [784, cap] sorted pos
        valid = arange_cap[None, :] < cnt[:, None]
        srcc = np.minimum(src, E - 1)
        edge_ids = np.where(valid, order[srcc], E)    # E -> pad row
        sort_ids = np.where(valid, srcc, E)           # for sorted-keyed arrays

        x_pad = senders_bf[edge_ids.reshape(-1)]      # [784*cap, 16] bf16
        l_pad = len_ext[edge_ids.reshape(-1)]         # pad row E -> 6.0
        r_pad = rk_ext[sort_ids.reshape(-1)]          # pad row E -> 0.0

        # [784, cb, 128, F] -> [blk, p, u, ph, f]
        x_t = x_pad.reshape(BLKS, UPB, cb, 128, F).transpose(0, 3, 1, 2, 4) \
            .reshape(BLKS, 128, T * F)
        # [784, cb, 128] -> [p, blk, u, ph]
        len_t = l_pad.reshape(BLKS, UPB, cb, 128).transpose(3, 0, 1, 2) \
            .reshape(128, nchunks)
        rk_t = r_pad.reshape(BLKS, UPB, cb, 128).transpose(3, 0, 1, 2) \
            .reshape(128, nchunks)
        in_maps.append({
            "x_t": np.ascontiguousarray(x_t),
            "len_t": np.ascontiguousarray(len_t),
            "rk_t": np.ascontiguousarray(rk_t),
        })
    return in_maps


def _get_program(cb: int, mode: str = "paged", reps: int = 1):
    key = (cb, reps)
    if key not in _CACHE:
        _CACHE[key] = _build_program(cb, reps)
    return _CACHE[key]


def _pick_cb(receivers):
    """Smallest chunks-per-bucket that fits the densest 16-node bucket."""
    counts = np.bincount(np.asarray(receivers).astype(np.int64) // W,
                         minlength=N_RBUCKETS_TOT)
    return max(5, int(-(-counts.max() // 128)))


def _run(inputs, cb=None, mode="paged", trace=False, **run_kwargs):
    from concourse.bass_utils import run_bass_kernel_spmd

    if cb is None:
        cb = _pick_cb(inputs["receivers"])
    in_maps = _prepare_inputs(inputs["senders"], inputs["lengths"],
                              inputs["receivers"], cb)
    nc = _get_program(cb)
    try:
        res = run_bass_kernel_spmd(nc, in_maps, core_ids=list(range(N_CORES)),
                                   trace=trace, **run_kwargs)
    except Exception:
        # transient NRT device wedges have been observed; one retry
        res = run_bass_kernel_spmd(nc, in_maps, core_ids=list(range(N_CORES)),
                                   trace=trace, **run_kwargs)
    out_full = np.zeros((E, F), np.float32)
    for k in range(N_CORES):
        nk = min(NODES_PER_CORE, N_NODES - NODES_PER_CORE * k)
        if nk <= 0:
            continue
        out_full[NODES_PER_CORE * k:NODES_PER_CORE * k + nk] = \
            res.results[k]["out"][:nk]
    return out_full, res


def kernel(senders, lengths, vectors, receivers):
    out, _ = _run({"senders": senders, "lengths": lengths,
                   "receivers": receivers})
    return out
